# revision 21
# baseline (speedup 1.0000x reference)
"""Trainium2 Bass kernel for nn_DenseDSnetwork (DeepSets-over-subgraphs GNN readout).

Self-contained: kernel(**inputs) takes the FULL unsharded inputs, shards
subgraphs across 8 NeuronCores (whole graphs stay on one core; subgraph_idx
is sorted), runs a Bass/Tile kernel per core via run_bass_kernel_spmd, and
gathers the full [4096, 10] output.

Per-core design (g_loc=512 graphs, nblk=4 blocks of GB=128 graphs):
 - h kept in SBUF as [D-part, rows] bf16, updated IN PLACE (both c-chunks'
   matmuls of a column group are issued before either ELU write).
 - segment-mean via matmul with a host-precomputed scaled one-hot
   A[row, g] = 1/count[g]: lhsT = h-rows tile (from a full-block DMA
   transpose), rhs = A tile, accumulating m^T = [D-part, G] directly in
   PSUM — no transpose/scale fixup.
 - x2^T = [G-part, Dout] directly via lhsT=m^T chunk, rhs=fcs_w row-block;
   scattered back to rows with the one-hot AT[g, row] as a third
   accumulating matmul into the x1 PSUM.
 - Linear bias pre-added to x2^T (the one-hot scatter then delivers +B to
   every row exactly once), so the ELU uses immediate-scalar ops only:
     e = exp(z); t = min(e-1, 0) (VectorE); h' = max(z,0)+t (VectorE).
 - A and AT are resident in SBUF across all 3 layers + head.
"""
import sys
sys.path.insert(0, "/opt/trn_rl_repo")


import math
from contextlib import ExitStack

import numpy as np

import concourse.bass as bass
import concourse.bacc as bacc
import concourse.mybir as mybir
import concourse.tile as tile

BF16 = mybir.dt.bfloat16
F32 = mybir.dt.float32
AF = mybir.ActivationFunctionType
ALU = mybir.AluOpType

GB = 128          # graphs per block (= PSUM partitions for m^T)
GRP = 512         # rows per main-pass group (= PSUM bank free size fp32)


def make_cfg(S, G, D, L, H, T, ncores):
    assert D == 256 and H == 2 * D, "kernel is specialized to D=256"
    g_loc = G // ncores
    nblk = g_loc // GB
    assert g_loc % GB == 0
    return dict(S=S, G=G, D=D, L=L, H=H, T=T, ncores=ncores,
                g_loc=g_loc, nblk=nblk)


def host_prep(inputs, cfg):
    """Split/pad/transpose inputs into per-core in_maps. Returns (in_maps, meta)."""
    S, G, D, L, T = cfg["S"], cfg["G"], cfg["D"], cfg["L"], cfg["T"]
    ncores, g_loc, nblk = cfg["ncores"], cfg["g_loc"], cfg["nblk"]
    bf = np.dtype(mybir.dt.np(BF16))

    h = np.ascontiguousarray(np.asarray(inputs["h_subgraph"], np.float32))
    idx = np.asarray(inputs["subgraph_idx"]).astype(np.int64)
    assert h.shape == (S, D)
    assert np.all(np.diff(idx) >= 0), "subgraph_idx must be sorted"

    counts = np.bincount(idx, minlength=G).astype(np.float32)
    inv = (1.0 / np.maximum(counts, 1.0)).astype(np.float32)

    # block row ranges: block (c,b) covers graphs [g0, g0+GB)
    nblk_tot = ncores * nblk
    g_edges = np.arange(nblk_tot + 1) * GB
    r_edges = np.searchsorted(idx, g_edges)          # row boundaries
    blk_rows = np.diff(r_edges)
    RB = 128 * int(math.ceil(blk_rows.max() / 128.0))
    W = nblk * RB

    fc_w = np.asarray(inputs["fc_w"], np.float32)
    fc_b = np.asarray(inputs["fc_b"], np.float32)
    fcs_w = np.asarray(inputs["fcs_w"], np.float32)
    fcs_b = np.asarray(inputs["fcs_b"], np.float32)
    f1_w = np.asarray(inputs["f1_w"], np.float32)
    f1_b = np.asarray(inputs["f1_b"], np.float32)
    f2_w = np.asarray(inputs["f2_w"], np.float32)
    f2_b = np.asarray(inputs["f2_b"], np.float32)

    # shared weight arrays
    fcwd = np.zeros((L, 2, 2, 128, 128), bf)         # [i,k,c] lhsT tiles
    fcswd = np.zeros((L, 2, 128, 256), bf)           # [i,k] rhs row-blocks
    bbcd = np.zeros((L, 128, 256), np.float32)   # bias broadcast across G
    for i in range(L):
        for k in range(2):
            for m in range(2):
                fcwd[i, k, m] = fc_w[i][128*k:128*k+128, 128*m:128*m+128].astype(bf)
            fcswd[i, k] = fcs_w[i][128*k:128*k+128, :].astype(bf)
        bbcd[i, :, :] = (fc_b[i] + fcs_b[i])[None, :]
    f1wd = np.zeros((2, 4, 128, 128), bf)
    f1bd = np.zeros((128, 4), np.float32)
    for k in range(2):
        for m in range(4):
            f1wd[k, m] = f1_w[128*k:128*k+128, 128*m:128*m+128].astype(bf)
    for m in range(4):
        f1bd[:, m] = f1_b[128*m:128*m+128]
    f2wd = np.zeros((4, 128, T), bf)
    for k in range(4):
        f2wd[k] = f2_w[128*k:128*k+128, :].astype(bf)
    f2bd = np.zeros((128, 1), np.float32)
    f2bd[:T, 0] = f2_b

    in_maps = []
    for c in range(ncores):
        hT = np.zeros((2, 128, W), bf)
        Ad = np.zeros((nblk, 128, RB), bf)     # [row-part, tile*128+g] = inv[g]
        ATd = np.zeros((nblk, 128, RB), bf)    # [g, row] one-hot
        for b in range(nblk):
            bi = c * nblk + b
            r0, r1 = int(r_edges[bi]), int(r_edges[bi + 1])
            n = r1 - r0
            rows = h[r0:r1].astype(bf).astype(np.float32)   # bf16-rounded
            for k in range(2):
                hT[k, :, b*RB:b*RB+n] = rows[:, 128*k:128*k+128].T.astype(bf)
            lb = (idx[r0:r1] - bi * GB).astype(np.int64)
            assert lb.min() >= 0 and lb.max() < GB
            j = np.arange(n)
            g0 = bi * GB
            Ad[b][j % 128, (j // 128) * 128 + lb] = inv[g0 + lb]
            ATd[b][lb, j] = 1.0
        in_maps.append(dict(hT=hT, Ad=Ad, ATd=ATd,
                            fcwd=fcwd, fcswd=fcswd,
                            bbcd=bbcd,
                            f1wd=f1wd, f1bd=f1bd, f2wd=f2wd, f2bd=f2bd))
    meta = dict(RB=RB, W=W, r_edges=r_edges)
    return in_maps, meta


def build(cfg, meta, bench_loop=False):
    L, T = cfg["L"], cfg["T"]
    g_loc, nblk = cfg["g_loc"], cfg["nblk"]
    RB, W = meta["RB"], meta["W"]
    ntile = RB // 128
    ngrp = (RB + GRP - 1) // GRP

    nc = bacc.Bacc("TRN2", target_bir_lowering=False, debug=False)

    hT_d = nc.dram_tensor("hT", [2, 128, W], BF16, kind="ExternalInput").ap()
    A_d = nc.dram_tensor("Ad", [nblk, 128, RB], BF16, kind="ExternalInput").ap()
    AT_d = nc.dram_tensor("ATd", [nblk, 128, RB], BF16, kind="ExternalInput").ap()
    fcw_d = nc.dram_tensor("fcwd", [L, 2, 2, 128, 128], BF16, kind="ExternalInput").ap()
    fcsw_d = nc.dram_tensor("fcswd", [L, 2, 128, 256], BF16, kind="ExternalInput").ap()
    bbc_d = nc.dram_tensor("bbcd", [L, 128, 256], F32, kind="ExternalInput").ap()
    f1w_d = nc.dram_tensor("f1wd", [2, 4, 128, 128], BF16, kind="ExternalInput").ap()
    f1b_d = nc.dram_tensor("f1bd", [128, 4], F32, kind="ExternalInput").ap()
    f2w_d = nc.dram_tensor("f2wd", [4, 128, T], BF16, kind="ExternalInput").ap()
    f2b_d = nc.dram_tensor("f2bd", [128, 1], F32, kind="ExternalInput").ap()
    out_d = nc.dram_tensor("outd", [T, g_loc], F32, kind="ExternalOutput").ap()
    niter_d = None
    if bench_loop:
        niter_d = nc.dram_tensor("niterd", [1, 1], mybir.dt.int32,
                                 kind="ExternalInput").ap()

    with tile.TileContext(nc) as tc, ExitStack() as ctx:
        hpool = ctx.enter_context(tc.tile_pool(name="h", bufs=1))
        wpool = ctx.enter_context(tc.tile_pool(name="w", bufs=1))
        hrpool = ctx.enter_context(tc.tile_pool(name="hr", bufs=2))
        tpool = ctx.enter_context(tc.tile_pool(name="t", bufs=1))
        x2pool = ctx.enter_context(tc.tile_pool(name="x2", bufs=2))
        epool = ctx.enter_context(tc.tile_pool(name="e", bufs=3))
        t2pool = ctx.enter_context(tc.tile_pool(name="t2", bufs=4))
        hidpool = ctx.enter_context(tc.tile_pool(name="hid", bufs=1))
        opool = ctx.enter_context(tc.tile_pool(name="o", bufs=1))
        # PSUM (8 banks): zps 3x2 | m_ps 1 | x2t 1
        ps_m = ctx.enter_context(tc.tile_pool(name="psm", bufs=1, space="PSUM"))
        ps_x2t = ctx.enter_context(tc.tile_pool(name="psx2t", bufs=1, space="PSUM"))
        ps_z = ctx.enter_context(tc.tile_pool(name="psz", bufs=3, space="PSUM"))

        def load(dst, src):
            nc.scalar.dma_start(dst, src)

        if bench_loop:
            from concourse.bass_types import RegisterHandles
            niter_sb = wpool.tile([1, 1], mybir.dt.int32, tag="niter", name="niter")
            nc.sync.dma_start(niter_sb[:], niter_d[:])
            _regs = []
            for _eng in (nc.sync, nc.scalar, nc.vector, nc.tensor, nc.gpsimd):
                _r = _eng.alloc_register(f"niter_{_eng.engine.name}")
                _eng.reg_load(_r, niter_sb[0:1, 0:1])
                _regs.append(_r)
            nval = nc.snap(RegisterHandles(_regs), min_val=1, max_val=100000)
            loop_cm = tc.For_i(0, nval, 1)
            loop_cm.__enter__()

        # --- persistent tensors; load order = per-block just-in-time ---
        a_sb = [wpool.tile([128, RB], BF16, tag=f"a{b}", name=f"a{b}")
                for b in range(nblk)]
        at_sb = [wpool.tile([128, RB], BF16, tag=f"at{b}", name=f"at{b}")
                 for b in range(nblk)]
        hbuf = {}
        for k in range(2):
            for b in range(nblk):
                hbuf[k, b] = hpool.tile([128, RB], BF16,
                                        tag=f"h{k}{b}", name=f"h{k}{b}")
        fcw_sb = [[[wpool.tile([128, 128], BF16, tag=f"fcw{i}{k}{m}", name=f"fcw{i}{k}{m}")
                    for m in range(2)] for k in range(2)] for i in range(L)]
        fcsw_sb = [[wpool.tile([128, 256], BF16, tag=f"fcsw{i}{k}", name=f"fcsw{i}{k}")
                    for k in range(2)] for i in range(L)]

        bbc_sb = [wpool.tile([128, 256], F32, tag=f"bbc{i}", name=f"bbc{i}")
                  for i in range(L)]
        f1w_sb = [[wpool.tile([128, 128], BF16, tag=f"f1w{k}{m}", name=f"f1w{k}{m}")
                   for m in range(4)] for k in range(2)]
        f2w_sb = [wpool.tile([128, T], BF16, tag=f"f2w{k}", name=f"f2w{k}") for k in range(4)]
        f1b_sb = wpool.tile([128, 4], F32, tag="f1b", name="f1b")
        f2b_sb = wpool.tile([128, 1], F32, tag="f2b", name="f2b")

        def load_block(b):
            for k in range(2):
                nc.sync.dma_start(hbuf[k, b][:], hT_d[k, :, b*RB:(b+1)*RB])
            load(a_sb[b][:], A_d[b])

        def load_at(b):
            nc.sync.dma_start(at_sb[b][:], AT_d[b])

        def load_tail(step):
            # spread the non-critical loads across the early pipeline
            if step == 0:
                load(bbc_sb[1][:], bbc_d[1])
                for k in range(2):
                    load(fcsw_sb[1][k][:], fcsw_d[1, k])
                    for m in range(2):
                        load(fcw_sb[1][k][m][:], fcw_d[1, k, m])
            elif step == 1:
                load(bbc_sb[2][:], bbc_d[2])
                for k in range(2):
                    load(fcsw_sb[2][k][:], fcsw_d[2, k])
                    for m in range(2):
                        load(fcw_sb[2][k][m][:], fcw_d[2, k, m])
            elif step == 2:
                for k in range(2):
                    for m in range(4):
                        load(f1w_sb[k][m][:], f1w_d[k, m])
                for k in range(4):
                    load(f2w_sb[k][:], f2w_d[k])
                load(f1b_sb[:], f1b_d[:])
                load(f2b_sb[:], f2b_d[:])

        # prologue: block-0 essentials, then stream the rest just-in-time
        load_block(0)
        load(bbc_sb[0][:], bbc_d[0])
        for k in range(2):
            load(fcsw_sb[0][k][:], fcsw_d[0, k])
            for m in range(2):
                load(fcw_sb[0][k][m][:], fcw_d[0, k, m])

        # ---- per-block helpers ----
        def seg_block(b, mT_sb, slabbed=False):
            """segment-MEAN^T of block b -> writes mT_sb[k][:, b*128:(b+1)*128].

            Transposes the whole block's h into row-major (one DMA per
            k-chunk; `slabbed` splits into per-group slabs so transposes
            overlap the producing main pass), then m^T = sum_t
            hr_tile[t]^T @ A_tile[t] with A pre-scaled by 1/count;
            [D_k, G] chunks side by side in PSUM.
            """
            hrbig = hrpool.tile([128, ntile * 256], BF16, tag="hr", name="hr")
            rview = hrbig.rearrange("p (t k2 d) -> p t k2 d", k2=2, d=128)
            for k in range(2):
                if slabbed:
                    for t0 in range(0, ntile, 4):
                        tn = min(4, ntile - t0)
                        nc.sync.dma_start_transpose(
                            rview[:, t0:t0+tn, k, :],
                            hbuf[k, b][:, t0*128:(t0+tn)*128])
                else:
                    nc.sync.dma_start_transpose(rview[:, :, k, :], hbuf[k, b][:])
            m_ps = ps_m.tile([128, 256], F32, tag="mps", name="mps")
            for k in range(2):
                for t in range(ntile):
                    nc.tensor.matmul(
                        m_ps[:, k*128:(k+1)*128],
                        lhsT=hrbig[:, t*256 + k*128: t*256 + (k+1)*128],
                        rhs=a_sb[b][:, t*128:(t+1)*128],
                        start=(t == 0), stop=(t == ntile - 1))
                nc.scalar.activation(mT_sb[k][:, b*128:(b+1)*128],
                                     m_ps[:, k*128:(k+1)*128], AF.Copy)

        def x2_block(i, b, mT_sb):
            """x2^T for block b: [G-part, 256 Dout] bf16, bias folded in."""
            x2t = ps_x2t.tile([128, 256], F32, tag="x2t", name="x2t")
            for k in range(2):
                nc.tensor.matmul(x2t[:],
                                 lhsT=mT_sb[k][:, b*128:(b+1)*128],
                                 rhs=fcsw_sb[i][k][:],
                                 start=(k == 0), stop=(k == 1))
            x2sb = x2pool.tile([128, 256], BF16, tag="x2sb", name="x2sb")
            nc.vector.tensor_add(x2sb[:], x2t[:], bbc_sb[i][:])
            return x2sb

        def main_block(i, b, x2sb):
            """h = ELU(fc_w^T h + (x2+B)^T scattered) for block b, in place.

            Column groups are processed in PAIRS spanning two PSUM banks so
            the elementwise stage runs half as many (2x-sized) ops — fewer
            cross-engine sync edges, better fixed-overhead amortization.
            """
            for j0 in range(0, ngrp, 2):
                c0 = j0 * GRP
                n1 = min(GRP, RB - c0)
                n2 = min(GRP, max(0, RB - c0 - GRP))
                span = n1 + n2
                # all four (c, group) matmul sets first: the in-place ELU
                # write must not overtake any matmul's read of these columns
                zl = []
                for c in range(2):
                    zps = ps_z.tile([128, 2 * GRP], F32, tag="zps", name="zps")
                    for off, nn in ((0, n1), (GRP, n2)):
                        if nn == 0:
                            continue
                        cc = c0 + off
                        for k in range(2):
                            nc.tensor.matmul(zps[:, off:off+nn],
                                             lhsT=fcw_sb[i][k][c][:],
                                             rhs=hbuf[k, b][:, cc:cc+nn],
                                             start=(k == 0), stop=False)
                        nc.tensor.matmul(zps[:, off:off+nn],
                                         lhsT=x2sb[:, 128*c:128*c+128],
                                         rhs=at_sb[b][:, cc:cc+nn],
                                         start=False, stop=True)
                    zl.append(zps)
                for c in range(2):
                    zps = zl[c]
                    e_sb = epool.tile([128, 2 * GRP], BF16, tag="esb", name="esb")
                    nc.scalar.activation(e_sb[:, :span], zps[:, :span], AF.Exp)
                    t_sb = t2pool.tile([128, 2 * GRP], BF16, tag="tsb", name="tsb")
                    nc.vector.tensor_scalar(t_sb[:, :span], e_sb[:, :span],
                                            -1.0, 0.0, ALU.add, ALU.min)
                    nc.vector.scalar_tensor_tensor(
                        hbuf[c, b][:, c0:c0+span],
                        zps[:, :span], 0.0, t_sb[:, :span], ALU.max, ALU.add)

        # --- flat software pipeline: after each main unit, issue the next
        #     seg task (its SP transpose then runs during this unit's mms)
        #     and the next block's loads (layer 0 only) ---
        mT_layers = {}

        def mT_for(i):
            if i not in mT_layers:
                mT_layers[i] = [tpool.tile([128, g_loc], BF16, tag=f"mT{k}",
                                           bufs=2, name=f"mT{i}{k}")
                                for k in range(2)]
            return mT_layers[i]

        main_units = [(i, b) for i in range(L) for b in range(nblk)]
        seg_tasks = main_units + [("h", b) for b in range(nblk)]
        seg_block(0, mT_for(0))
        load_at(0)
        load_block(1)
        load_at(1)
        seg_ptr = 1
        for u, (i, b) in enumerate(main_units):
            x2sb = x2_block(i, b, mT_for(i))
            main_block(i, b, x2sb)
            if u + 2 < nblk:            # keep one block of load lead
                load_block(u + 2)
                load_at(u + 2)
            load_tail(u - (nblk - 2))
            # double lookahead in the last layer drains the head segs early
            for _ in range(2 if i == L - 1 else 1):
                if seg_ptr < len(seg_tasks):
                    si, sb2 = seg_tasks[seg_ptr]
                    seg_ptr += 1
                    seg_block(sb2, mT_for(si),
                              slabbed=(seg_ptr == len(seg_tasks)))
        while seg_ptr < len(seg_tasks):
            si, sb2 = seg_tasks[seg_ptr]
            seg_ptr += 1
            seg_block(sb2, mT_for(si), slabbed=(seg_ptr == len(seg_tasks)))

        # --- head: f1 per block as each mT column-block lands ---
        mT_sb = mT_for("h")
        hid_tiles = [ps_z.tile([128, 2 * GRP], F32, tag="zps", name=f"hidps{mm}")
                     for mm in range(2)]
        hid_ps = [hid_tiles[m // 2][:, (m % 2) * g_loc:(m % 2) * g_loc + g_loc]
                  for m in range(4)]
        for b in range(nblk):
            for m in range(4):
                for k in range(2):
                    nc.tensor.matmul(hid_ps[m][:, b*128:(b+1)*128],
                                     lhsT=f1w_sb[k][m][:],
                                     rhs=mT_sb[k][:, b*128:(b+1)*128],
                                     start=(k == 0), stop=(k == 1))
        hid_sb = []
        for m in range(4):
            hs = hidpool.tile([128, g_loc], BF16, tag=f"hid{m}", name=f"hid{m}")
            nc.scalar.activation(hs[:], hid_ps[m][:], AF.Relu,
                                 bias=f1b_sb[:, m:m+1])
            hid_sb.append(hs)
        out_ps = ps_z.tile([128, 2 * GRP], F32, tag="zps", name="outps")
        for k in range(4):
            nc.tensor.matmul(out_ps[0:T, :g_loc], lhsT=f2w_sb[k][:, 0:T],
                             rhs=hid_sb[k][:], start=(k == 0), stop=(k == 3))
        out_sb = opool.tile([128, g_loc], F32, tag="outsb", name="outsb")
        nc.vector.tensor_scalar_add(out_sb[0:T, :], out_ps[0:T, :g_loc], f2b_sb[0:T, 0:1])
        load(out_d[:, :], out_sb[0:T, :])
        if bench_loop:
            loop_cm.__exit__(None, None, None)

    nc.finalize()
    return nc


def unshard(results, cfg):
    """per-core outd [T, g_loc] -> full [G, T] fp32."""
    outs = [np.asarray(r["outd"]).T for r in results]   # [g_loc, T] each
    return np.concatenate(outs, axis=0).astype(np.float32)


_NCORES = 8


def kernel(**inputs):
    h = np.asarray(inputs["h_subgraph"])
    S, D = h.shape
    cfg = make_cfg(S=S, G=4096, D=D, L=3, H=2 * D, T=10, ncores=_NCORES)
    in_maps, meta = host_prep(inputs, cfg)
    nc = build(cfg, meta, bench_loop=False)
    from concourse import bass_utils
    res = bass_utils.run_bass_kernel_spmd(nc, in_maps, core_ids=list(range(_NCORES)))
    return unshard(res.results, cfg)


# revision 24
# speedup vs baseline: 1.0020x; 1.0020x over previous
"""Trainium2 Bass kernel for nn_DenseDSnetwork (DeepSets-over-subgraphs GNN readout).

Self-contained: kernel(**inputs) takes the FULL unsharded inputs, shards
subgraphs across 8 NeuronCores (whole graphs stay on one core; subgraph_idx
is sorted), runs a Bass/Tile kernel per core via run_bass_kernel_spmd, and
gathers the full [4096, 10] output.

Per-core design (g_loc=512 graphs, nblk=4 blocks of GB=128 graphs):
 - h kept in SBUF as [D-part, rows] bf16, updated IN PLACE (both c-chunks'
   matmuls of a column group are issued before either ELU write).
 - segment-mean via matmul with a host-precomputed scaled one-hot
   A[row, g] = 1/count[g]: lhsT = h-rows tile (from a full-block DMA
   transpose), rhs = A tile, accumulating m^T = [D-part, G] directly in
   PSUM — no transpose/scale fixup.
 - x2^T = [G-part, Dout] directly via lhsT=m^T chunk, rhs=fcs_w row-block;
   scattered back to rows with the one-hot AT[g, row] as a third
   accumulating matmul into the x1 PSUM.
 - Linear bias pre-added to x2^T (the one-hot scatter then delivers +B to
   every row exactly once), so the ELU uses immediate-scalar ops only:
     e = exp(z); t = min(e-1, 0) (VectorE); h' = max(z,0)+t (VectorE).
 - A and AT are resident in SBUF across all 3 layers + head.
"""
import sys
sys.path.insert(0, "/opt/trn_rl_repo")


import math
from contextlib import ExitStack

import numpy as np

import concourse.bass as bass
import concourse.bacc as bacc
import concourse.mybir as mybir
import concourse.tile as tile

BF16 = mybir.dt.bfloat16
F32 = mybir.dt.float32
AF = mybir.ActivationFunctionType
ALU = mybir.AluOpType

GB = 128          # graphs per block (= PSUM partitions for m^T)
GRP = 512         # rows per main-pass group (= PSUM bank free size fp32)


def make_cfg(S, G, D, L, H, T, ncores):
    assert D == 256 and H == 2 * D, "kernel is specialized to D=256"
    g_loc = G // ncores
    nblk = g_loc // GB
    assert g_loc % GB == 0
    return dict(S=S, G=G, D=D, L=L, H=H, T=T, ncores=ncores,
                g_loc=g_loc, nblk=nblk)


def host_prep(inputs, cfg):
    """Split/pad/transpose inputs into per-core in_maps. Returns (in_maps, meta)."""
    S, G, D, L, T = cfg["S"], cfg["G"], cfg["D"], cfg["L"], cfg["T"]
    ncores, g_loc, nblk = cfg["ncores"], cfg["g_loc"], cfg["nblk"]
    bf = np.dtype(mybir.dt.np(BF16))

    h = np.ascontiguousarray(np.asarray(inputs["h_subgraph"], np.float32))
    idx = np.asarray(inputs["subgraph_idx"]).astype(np.int64)
    assert h.shape == (S, D)
    assert np.all(np.diff(idx) >= 0), "subgraph_idx must be sorted"

    counts = np.bincount(idx, minlength=G).astype(np.float32)
    inv = (1.0 / np.maximum(counts, 1.0)).astype(np.float32)

    # block row ranges: block (c,b) covers graphs [g0, g0+GB)
    nblk_tot = ncores * nblk
    g_edges = np.arange(nblk_tot + 1) * GB
    r_edges = np.searchsorted(idx, g_edges)          # row boundaries
    blk_rows = np.diff(r_edges)
    RB = 128 * int(math.ceil(blk_rows.max() / 128.0))
    W = nblk * RB

    fc_w = np.asarray(inputs["fc_w"], np.float32)
    fc_b = np.asarray(inputs["fc_b"], np.float32)
    fcs_w = np.asarray(inputs["fcs_w"], np.float32)
    fcs_b = np.asarray(inputs["fcs_b"], np.float32)
    f1_w = np.asarray(inputs["f1_w"], np.float32)
    f1_b = np.asarray(inputs["f1_b"], np.float32)
    f2_w = np.asarray(inputs["f2_w"], np.float32)
    f2_b = np.asarray(inputs["f2_b"], np.float32)

    # shared weight arrays
    fcwd = np.zeros((L, 2, 2, 128, 128), bf)         # [i,k,c] lhsT tiles
    fcswd = np.zeros((L, 2, 128, 256), bf)           # [i,k] rhs row-blocks
    bbcd = np.zeros((L, 128, 256), np.float32)   # bias broadcast across G
    for i in range(L):
        for k in range(2):
            for m in range(2):
                fcwd[i, k, m] = fc_w[i][128*k:128*k+128, 128*m:128*m+128].astype(bf)
            fcswd[i, k] = fcs_w[i][128*k:128*k+128, :].astype(bf)
        bbcd[i, :, :] = (fc_b[i] + fcs_b[i])[None, :]
    f1wd = np.zeros((2, 4, 128, 128), bf)
    f1bd = np.zeros((128, 4), np.float32)
    for k in range(2):
        for m in range(4):
            f1wd[k, m] = f1_w[128*k:128*k+128, 128*m:128*m+128].astype(bf)
    for m in range(4):
        f1bd[:, m] = f1_b[128*m:128*m+128]
    f2wd = np.zeros((4, 128, T), bf)
    for k in range(4):
        f2wd[k] = f2_w[128*k:128*k+128, :].astype(bf)
    f2bd = np.zeros((128, 1), np.float32)
    f2bd[:T, 0] = f2_b

    in_maps = []
    for c in range(ncores):
        hT = np.zeros((2, 128, W), bf)
        Ad = np.zeros((nblk, 128, RB), bf)     # [row-part, tile*128+g] = inv[g]
        ATd = np.zeros((nblk, 128, RB), bf)    # [g, row] one-hot
        for b in range(nblk):
            bi = c * nblk + b
            r0, r1 = int(r_edges[bi]), int(r_edges[bi + 1])
            n = r1 - r0
            rows = h[r0:r1].astype(bf).astype(np.float32)   # bf16-rounded
            for k in range(2):
                hT[k, :, b*RB:b*RB+n] = rows[:, 128*k:128*k+128].T.astype(bf)
            lb = (idx[r0:r1] - bi * GB).astype(np.int64)
            assert lb.min() >= 0 and lb.max() < GB
            j = np.arange(n)
            g0 = bi * GB
            Ad[b][j % 128, (j // 128) * 128 + lb] = inv[g0 + lb]
            ATd[b][lb, j] = 1.0
        in_maps.append(dict(hT=hT, Ad=Ad, ATd=ATd,
                            fcwd=fcwd, fcswd=fcswd,
                            bbcd=bbcd,
                            f1wd=f1wd, f1bd=f1bd, f2wd=f2wd, f2bd=f2bd))
    meta = dict(RB=RB, W=W, r_edges=r_edges)
    return in_maps, meta


def build(cfg, meta, bench_loop=False):
    L, T = cfg["L"], cfg["T"]
    g_loc, nblk = cfg["g_loc"], cfg["nblk"]
    RB, W = meta["RB"], meta["W"]
    ntile = RB // 128
    ngrp = (RB + GRP - 1) // GRP

    nc = bacc.Bacc("TRN2", target_bir_lowering=False, debug=False)

    hT_d = nc.dram_tensor("hT", [2, 128, W], BF16, kind="ExternalInput").ap()
    A_d = nc.dram_tensor("Ad", [nblk, 128, RB], BF16, kind="ExternalInput").ap()
    AT_d = nc.dram_tensor("ATd", [nblk, 128, RB], BF16, kind="ExternalInput").ap()
    fcw_d = nc.dram_tensor("fcwd", [L, 2, 2, 128, 128], BF16, kind="ExternalInput").ap()
    fcsw_d = nc.dram_tensor("fcswd", [L, 2, 128, 256], BF16, kind="ExternalInput").ap()
    bbc_d = nc.dram_tensor("bbcd", [L, 128, 256], F32, kind="ExternalInput").ap()
    f1w_d = nc.dram_tensor("f1wd", [2, 4, 128, 128], BF16, kind="ExternalInput").ap()
    f1b_d = nc.dram_tensor("f1bd", [128, 4], F32, kind="ExternalInput").ap()
    f2w_d = nc.dram_tensor("f2wd", [4, 128, T], BF16, kind="ExternalInput").ap()
    f2b_d = nc.dram_tensor("f2bd", [128, 1], F32, kind="ExternalInput").ap()
    out_d = nc.dram_tensor("outd", [T, g_loc], F32, kind="ExternalOutput").ap()
    niter_d = None
    if bench_loop:
        niter_d = nc.dram_tensor("niterd", [1, 1], mybir.dt.int32,
                                 kind="ExternalInput").ap()

    with tile.TileContext(nc) as tc, ExitStack() as ctx:
        hpool = ctx.enter_context(tc.tile_pool(name="h", bufs=1))
        wpool = ctx.enter_context(tc.tile_pool(name="w", bufs=1))
        hrpool = ctx.enter_context(tc.tile_pool(name="hr", bufs=2))
        tpool = ctx.enter_context(tc.tile_pool(name="t", bufs=1))
        x2pool = ctx.enter_context(tc.tile_pool(name="x2", bufs=2))
        epool = ctx.enter_context(tc.tile_pool(name="e", bufs=3))
        t2pool = ctx.enter_context(tc.tile_pool(name="t2", bufs=4))
        hidpool = ctx.enter_context(tc.tile_pool(name="hid", bufs=1))
        opool = ctx.enter_context(tc.tile_pool(name="o", bufs=1))
        # PSUM (8 banks): zps 5 | m_ps 2 | x2t 1
        ps_m = ctx.enter_context(tc.tile_pool(name="psm", bufs=2, space="PSUM"))
        ps_x2t = ctx.enter_context(tc.tile_pool(name="psx2t", bufs=1, space="PSUM"))
        ps_z = ctx.enter_context(tc.tile_pool(name="psz", bufs=5, space="PSUM"))

        def load(dst, src):
            nc.scalar.dma_start(dst, src)

        if bench_loop:
            from concourse.bass_types import RegisterHandles
            niter_sb = wpool.tile([1, 1], mybir.dt.int32, tag="niter", name="niter")
            nc.sync.dma_start(niter_sb[:], niter_d[:])
            _regs = []
            for _eng in (nc.sync, nc.scalar, nc.vector, nc.tensor, nc.gpsimd):
                _r = _eng.alloc_register(f"niter_{_eng.engine.name}")
                _eng.reg_load(_r, niter_sb[0:1, 0:1])
                _regs.append(_r)
            nval = nc.snap(RegisterHandles(_regs), min_val=1, max_val=100000)
            loop_cm = tc.For_i(0, nval, 1)
            loop_cm.__enter__()

        # --- persistent tensors; load order = per-block just-in-time ---
        a_sb = [wpool.tile([128, RB], BF16, tag=f"a{b}", name=f"a{b}")
                for b in range(nblk)]
        at_sb = [wpool.tile([128, RB], BF16, tag=f"at{b}", name=f"at{b}")
                 for b in range(nblk)]
        hbuf = {}
        for k in range(2):
            for b in range(nblk):
                hbuf[k, b] = hpool.tile([128, RB], BF16,
                                        tag=f"h{k}{b}", name=f"h{k}{b}")
        fcw_sb = [[[wpool.tile([128, 128], BF16, tag=f"fcw{i}{k}{m}", name=f"fcw{i}{k}{m}")
                    for m in range(2)] for k in range(2)] for i in range(L)]
        fcsw_sb = [[wpool.tile([128, 256], BF16, tag=f"fcsw{i}{k}", name=f"fcsw{i}{k}")
                    for k in range(2)] for i in range(L)]

        bbc_sb = [wpool.tile([128, 256], F32, tag=f"bbc{i}", name=f"bbc{i}")
                  for i in range(L)]
        f1w_sb = [[wpool.tile([128, 128], BF16, tag=f"f1w{k}{m}", name=f"f1w{k}{m}")
                   for m in range(4)] for k in range(2)]
        f2w_sb = [wpool.tile([128, T], BF16, tag=f"f2w{k}", name=f"f2w{k}") for k in range(4)]
        f1b_sb = wpool.tile([128, 4], F32, tag="f1b", name="f1b")
        f2b_sb = wpool.tile([128, 1], F32, tag="f2b", name="f2b")

        def load_block(b):
            for k in range(2):
                nc.sync.dma_start(hbuf[k, b][:], hT_d[k, :, b*RB:(b+1)*RB])
            load(a_sb[b][:], A_d[b])

        def load_at(b):
            nc.sync.dma_start(at_sb[b][:], AT_d[b])

        def load_tail(step):
            # spread the non-critical loads across the early pipeline
            if step == 0:
                load(bbc_sb[1][:], bbc_d[1])
                for k in range(2):
                    load(fcsw_sb[1][k][:], fcsw_d[1, k])
                    for m in range(2):
                        load(fcw_sb[1][k][m][:], fcw_d[1, k, m])
            elif step == 1:
                load(bbc_sb[2][:], bbc_d[2])
                for k in range(2):
                    load(fcsw_sb[2][k][:], fcsw_d[2, k])
                    for m in range(2):
                        load(fcw_sb[2][k][m][:], fcw_d[2, k, m])
            elif step == 2:
                for k in range(2):
                    for m in range(4):
                        load(f1w_sb[k][m][:], f1w_d[k, m])
                for k in range(4):
                    load(f2w_sb[k][:], f2w_d[k])
                load(f1b_sb[:], f1b_d[:])
                load(f2b_sb[:], f2b_d[:])

        # prologue: block-0 essentials, then stream the rest just-in-time
        load_block(0)
        load(bbc_sb[0][:], bbc_d[0])
        for k in range(2):
            load(fcsw_sb[0][k][:], fcsw_d[0, k])
            for m in range(2):
                load(fcw_sb[0][k][m][:], fcw_d[0, k, m])

        # ---- per-block helpers ----
        def seg_block(b, mT_sb, slabbed=False):
            """segment-MEAN^T of block b -> writes mT_sb[k][:, b*128:(b+1)*128].

            Transposes the whole block's h into row-major (one DMA per
            k-chunk; `slabbed` splits into per-group slabs so transposes
            overlap the producing main pass), then m^T = sum_t
            hr_tile[t]^T @ A_tile[t] with A pre-scaled by 1/count;
            [D_k, G] chunks side by side in PSUM.
            """
            hrbig = hrpool.tile([128, ntile * 256], BF16, tag="hr", name="hr")
            rview = hrbig.rearrange("p (t k2 d) -> p t k2 d", k2=2, d=128)
            for k in range(2):
                if slabbed:
                    for t0 in range(0, ntile, 4):
                        tn = min(4, ntile - t0)
                        nc.sync.dma_start_transpose(
                            rview[:, t0:t0+tn, k, :],
                            hbuf[k, b][:, t0*128:(t0+tn)*128])
                else:
                    nc.sync.dma_start_transpose(rview[:, :, k, :], hbuf[k, b][:])
            m_ps = ps_m.tile([128, 256], F32, tag="mps", name="mps")
            for k in range(2):
                for t in range(ntile):
                    nc.tensor.matmul(
                        m_ps[:, k*128:(k+1)*128],
                        lhsT=hrbig[:, t*256 + k*128: t*256 + (k+1)*128],
                        rhs=a_sb[b][:, t*128:(t+1)*128],
                        start=(t == 0), stop=(t == ntile - 1))
                nc.scalar.activation(mT_sb[k][:, b*128:(b+1)*128],
                                     m_ps[:, k*128:(k+1)*128], AF.Copy)

        def x2_block(i, b, mT_sb):
            """x2^T for block b: [G-part, 256 Dout] bf16, bias folded in."""
            x2t = ps_x2t.tile([128, 256], F32, tag="x2t", name="x2t")
            for k in range(2):
                nc.tensor.matmul(x2t[:],
                                 lhsT=mT_sb[k][:, b*128:(b+1)*128],
                                 rhs=fcsw_sb[i][k][:],
                                 start=(k == 0), stop=(k == 1))
            x2sb = x2pool.tile([128, 256], BF16, tag="x2sb", name="x2sb")
            nc.vector.tensor_add(x2sb[:], x2t[:], bbc_sb[i][:])
            return x2sb

        def main_block(i, b, x2sb):
            """h = ELU(fc_w^T h + (x2+B)^T scattered) for block b, in place."""
            for j in range(ngrp):
                c0 = j * GRP
                n = min(GRP, RB - c0)
                # both c-units' matmuls first: the in-place ELU write of c=0
                # must not overtake the c=1 matmul's read of the same columns
                zl = []
                for c in range(2):
                    zps = ps_z.tile([128, GRP], F32, tag="zps", name="zps")
                    for k in range(2):
                        nc.tensor.matmul(zps[:, :n], lhsT=fcw_sb[i][k][c][:],
                                         rhs=hbuf[k, b][:, c0:c0+n],
                                         start=(k == 0), stop=False)
                    nc.tensor.matmul(zps[:, :n],
                                     lhsT=x2sb[:, 128*c:128*c+128],
                                     rhs=at_sb[b][:, c0:c0+n], start=False, stop=True)
                    zl.append(zps)
                for c in range(2):
                    zps = zl[c]
                    e_sb = epool.tile([128, GRP], BF16, tag="esb", name="esb")
                    nc.scalar.activation(e_sb[:, :n], zps[:, :n], AF.Exp)
                    t_sb = t2pool.tile([128, GRP], BF16, tag="tsb", name="tsb")
                    nc.vector.tensor_scalar(t_sb[:, :n], e_sb[:, :n],
                                            -1.0, 0.0, ALU.add, ALU.min)
                    nc.vector.scalar_tensor_tensor(
                        hbuf[c, b][:, c0:c0+n],
                        zps[:, :n], 0.0, t_sb[:, :n], ALU.max, ALU.add)

        # --- flat software pipeline: after each main unit, issue the next
        #     seg task (its SP transpose then runs during this unit's mms)
        #     and the next block's loads (layer 0 only) ---
        mT_layers = {}

        def mT_for(i):
            if i not in mT_layers:
                mT_layers[i] = [tpool.tile([128, g_loc], BF16, tag=f"mT{k}",
                                           bufs=2, name=f"mT{i}{k}")
                                for k in range(2)]
            return mT_layers[i]

        main_units = [(i, b) for i in range(L) for b in range(nblk)]
        seg_tasks = main_units + [("h", b) for b in range(nblk)]
        seg_block(0, mT_for(0))
        load_at(0)
        load_block(1)
        load_at(1)
        seg_ptr = 1
        for u, (i, b) in enumerate(main_units):
            x2sb = x2_block(i, b, mT_for(i))
            main_block(i, b, x2sb)
            if u + 2 < nblk:            # keep one block of load lead
                load_block(u + 2)
                load_at(u + 2)
            load_tail(u - (nblk - 2))
            # double lookahead in the last layer drains the head segs early
            for _ in range(2 if i == L - 1 else 1):
                if seg_ptr < len(seg_tasks):
                    si, sb2 = seg_tasks[seg_ptr]
                    seg_ptr += 1
                    seg_block(sb2, mT_for(si),
                              slabbed=(seg_ptr == len(seg_tasks)))
        while seg_ptr < len(seg_tasks):
            si, sb2 = seg_tasks[seg_ptr]
            seg_ptr += 1
            seg_block(sb2, mT_for(si), slabbed=(seg_ptr == len(seg_tasks)))

        # --- head: f1 per block as each mT column-block lands ---
        mT_sb = mT_for("h")
        hid_ps = [ps_z.tile([128, g_loc], F32, tag="zps", name=f"hidps{m}")
                  for m in range(4)]
        for b in range(nblk):
            for m in range(4):
                for k in range(2):
                    nc.tensor.matmul(hid_ps[m][:, b*128:(b+1)*128],
                                     lhsT=f1w_sb[k][m][:],
                                     rhs=mT_sb[k][:, b*128:(b+1)*128],
                                     start=(k == 0), stop=(k == 1))
        hid_sb = []
        for m in range(4):
            hs = hidpool.tile([128, g_loc], BF16, tag=f"hid{m}", name=f"hid{m}")
            nc.scalar.activation(hs[:], hid_ps[m][:], AF.Relu,
                                 bias=f1b_sb[:, m:m+1])
            hid_sb.append(hs)
        out_ps = ps_z.tile([128, g_loc], F32, tag="zps", name="outps")
        for k in range(4):
            nc.tensor.matmul(out_ps[0:T, :], lhsT=f2w_sb[k][:, 0:T],
                             rhs=hid_sb[k][:], start=(k == 0), stop=(k == 3))
        out_sb = opool.tile([128, g_loc], F32, tag="outsb", name="outsb")
        nc.vector.tensor_scalar_add(out_sb[0:T, :], out_ps[0:T, :], f2b_sb[0:T, 0:1])
        load(out_d[:, :], out_sb[0:T, :])
        if bench_loop:
            loop_cm.__exit__(None, None, None)

    nc.finalize()
    return nc


def unshard(results, cfg):
    """per-core outd [T, g_loc] -> full [G, T] fp32."""
    outs = [np.asarray(r["outd"]).T for r in results]   # [g_loc, T] each
    return np.concatenate(outs, axis=0).astype(np.float32)


_NCORES = 8


def kernel(**inputs):
    h = np.asarray(inputs["h_subgraph"])
    S, D = h.shape
    cfg = make_cfg(S=S, G=4096, D=D, L=3, H=2 * D, T=10, ncores=_NCORES)
    in_maps, meta = host_prep(inputs, cfg)
    nc = build(cfg, meta, bench_loop=False)
    from concourse import bass_utils
    res = bass_utils.run_bass_kernel_spmd(nc, in_maps, core_ids=list(range(_NCORES)))
    return unshard(res.results, cfg)
